# revision 2
# baseline (speedup 1.0000x reference)
"""Trainium2 Bass kernel for truncated BCH on 3D vector fields.

Math (matches the jax reference): with u = l + r, w = 0.125*(l - r):
  out_i = u_i + sum_j [ (D_j w_i) * u_j  +  (D~_j u_i) * w_j ]
where D_j v = v[.+1] - v[.-1] along spatial axis j (circulant wrap) and
D~ is the reversed diff (v[.-1] - v[.+1]), folding all signs so every
term is ADDED.  The 0.25 bracket scale is folded into w on the host
(0.125 per the u/w identity), so the device does no scaling at all.

Sharding: 8 cores = 2 batches x 4 X-slabs of 32 planes (+1 halo plane
each side, wrapped).  Host lays data per core as (D, Y, 34, 130) fp16
(z padded [z127 | z | z0]); output is (Y, D, 32, 128) fp16, cast to
fp32 on the host.

Per-core engine split (v1 CoreSim cost model, per-col ns):
  PE    (0.417/col): Y-diffs as circulant matmuls (lhsT = DyT for the
        w-side, -DyT for the u-side) + 6 identity-weight injections per
        (chunk, channel) into a PSUM accumulator.
  DVE   (0.521/col fp16 2x): x/z diffs + products p0 (dxw*u0),
        p1 (dxu~*w0), most of p4 (dzw*u2).
  Pool  (0.833/col): dy products p2/p3 straight out of PSUM (no
        evacuation!), p5 (dzu~*w2), q23 = p2+p3 pre-sum, p4 remainder.
  Act   (0.833/col+185): one PSUM->SBUF fp16 copy per chunk (all 3
        channels at once) + the w-tensor input DMAs.
  SP    : u-tensor input DMAs + output DMAs.
Output DMA uses a (y d)-major DRAM layout so dim0 = 384.
"""

import sys

sys.path.insert(0, "/opt/trn_rl_repo")

import numpy as np

import concourse.bass as bass
import concourse.bacc as bacc
import concourse.mybir as mybir
import concourse.tile as tile
from concourse.bass_utils import run_bass_kernel_spmd

B, D, X, Y, Z = 2, 3, 128, 128, 128
NCORES = 8
XS = (B * X) // NCORES  # 32 output x-planes per core
ZP = Z + 2              # z padded: [z127, z0..z127, z0]
KX = 2                  # x-planes per psum chunk (half-bank acc groups)
KB = 16                 # x-planes per DVE diff/product instruction

F16 = mybir.dt.float16
F32 = mybir.dt.float32


def _make_wmats() -> np.ndarray:
    """[DyT | -DyT | I] as one (Y, 3Y) fp16 matrix (lhsT layout).

    matmul(out, lhsT, rhs) computes lhsT.T @ rhs.  We want Dy @ v with
    Dy[y, y'] = delta(y'=y+1) - delta(y'=y-1) (wrap), so lhsT = Dy.T.
    """
    e = np.eye(Y, dtype=np.float32)
    dy = np.roll(e, -1, axis=0) - np.roll(e, 1, axis=0)
    dyt = dy.T
    mats = np.concatenate([dyt, -dyt, e], axis=1)
    return mats.astype(np.float16)


def build_nc(xs: int = XS, *, dbufs: int = 3, pbufs: int = 3, ybufs: int = 4,
             accbufs: int = 2, sbufs: int = 3, p4_dve: int = 5,
             q23_dve: int = 0) -> bass.Bass:
    xh = xs + 2
    nchunk = xs // KX
    nsuper = xs // KB if xs >= KB else 1
    kb = min(KB, xs)
    nc = bacc.Bacc(None)

    u_h = nc.declare_dram_parameter("u", [D, Y, xh, ZP], F16, isOutput=False)
    w_h = nc.declare_dram_parameter("w", [D, Y, xh, ZP], F16, isOutput=False)
    wm_h = nc.declare_dram_parameter("wmats", [Y, 3 * Y], F16, isOutput=False)
    out_h = nc.declare_dram_parameter("out", [Y, D, xs, Z], F16, isOutput=True)

    # (y d)-major view: dim0 = Y*D = 384, free = (x z) contiguous runs
    out_dram = out_h[:, :, :, :].rearrange("y d x z -> (y d) x z")

    with tile.TileContext(nc) as tc:
        with (
            tc.tile_pool(name="inp", bufs=1) as inp,
            tc.tile_pool(name="wp", bufs=1) as wp,
            tc.tile_pool(name="dpool", bufs=dbufs) as dpool,
            tc.tile_pool(name="ppool", bufs=pbufs) as ppool,
            tc.tile_pool(name="psum_dy", bufs=ybufs, space="PSUM") as psum_dy,
            tc.tile_pool(name="psum_acc", bufs=accbufs, space="PSUM") as psum_acc,
            tc.tile_pool(name="spool", bufs=sbufs) as spool,
        ):
            wt_m = wp.tile([Y, 3 * Y], F16, name="wt_m")
            nc.sync.dma_start(out=wt_m[:, :], in_=wm_h[:, :])
            dyT = wt_m[:, 0:Y]
            ndyT = wt_m[:, Y : 2 * Y]
            eyeT = wt_m[:, 2 * Y : 3 * Y]

            # Load channels in x-splits so early chunks start while the rest
            # streams.  u loads on SP, w loads on Act (2 parallel queues).
            cuts = [0, 6, 14, 22, xh] if xh >= 24 else [0, xh]
            ut, wt = [], []
            for i in range(D):
                ut.append(inp.tile([Y, xh, ZP], F16, name=f"ut{i}", tag=f"ut{i}"))
            for i in range(D):
                wt.append(inp.tile([Y, xh, ZP], F16, name=f"wt{i}", tag=f"wt{i}"))
            for a, b2 in zip(cuts, cuts[1:]):
                for i in range(D):
                    nc.sync.dma_start(out=ut[i][:, a:b2, :],
                                      in_=u_h[i, :, a:b2, :])
                    nc.scalar.dma_start(out=wt[i][:, a:b2, :],
                                        in_=w_h[i, :, a:b2, :])

            zc = slice(1, 1 + Z)       # center z view
            zp1 = slice(2, 2 + Z)      # z+1
            zm1 = slice(0, 0 + Z)      # z-1

            # Prime PE's vector clock against every input DMA with tiny
            # matmuls, so real matmuls never need a second (DMA) wait —
            # TRN2 matmul instructions support a single sync wait.
            scratch = psum_dy.tile([8, 8], F32, name="scratch", tag="dy")
            for a in cuts[:-1]:
                for t in ut + wt:
                    nc.tensor.matmul(scratch[:, 0:1], wt_m[:, 0:8],
                                     t[:, a : a + 1, 0:1], start=True, stop=True)

            # ---- DVE/Pool super-chunk stage: diffs + x/z products ----
            # per super-chunk s and channel i, produce [Y, kb, Z] fp16 tiles
            def stage_diffs(s):
                u0 = 1 + kb * s
                xsl = slice(u0, u0 + kb)
                xp1 = slice(u0 + 1, u0 + 1 + kb)
                xm1 = slice(u0 - 1, u0 - 1 + kb)
                prods = []
                for i in range(D):
                    dxw = dpool.tile([Y, kb, Z], F16, name="dxw", tag="dxw")
                    nc.vector.tensor_sub(out=dxw[:, :, :],
                                         in0=wt[i][:, xp1, zc],
                                         in1=wt[i][:, xm1, zc])
                    dxu = dpool.tile([Y, kb, Z], F16, name="dxu", tag="dxu")
                    nc.vector.tensor_sub(out=dxu[:, :, :],
                                         in0=ut[i][:, xm1, zc],
                                         in1=ut[i][:, xp1, zc])
                    dzw = dpool.tile([Y, kb, Z], F16, name="dzw", tag="dzw")
                    nc.vector.tensor_sub(out=dzw[:, :, :],
                                         in0=wt[i][:, xsl, zp1],
                                         in1=wt[i][:, xsl, zm1])
                    dzu = dpool.tile([Y, kb, Z], F16, name="dzu", tag="dzu")
                    nc.vector.tensor_sub(out=dzu[:, :, :],
                                         in0=ut[i][:, xsl, zm1],
                                         in1=ut[i][:, xsl, zp1])

                    p0 = ppool.tile([Y, kb, Z], F16, name="p0", tag="p0")
                    nc.vector.tensor_mul(out=p0[:, :, :], in0=dxw[:, :, :],
                                         in1=ut[0][:, xsl, zc])
                    p1 = ppool.tile([Y, kb, Z], F16, name="p1", tag="p1")
                    nc.vector.tensor_mul(out=p1[:, :, :], in0=dxu[:, :, :],
                                         in1=wt[0][:, xsl, zc])
                    # p4 engine split is a tuned balance knob
                    p4_eng = nc.vector if (s * D + i) < p4_dve else nc.gpsimd
                    p4 = ppool.tile([Y, kb, Z], F16, name="p4", tag="p4")
                    p4_eng.tensor_mul(out=p4[:, :, :], in0=dzw[:, :, :],
                                      in1=ut[2][:, xsl, zc])
                    p5 = ppool.tile([Y, kb, Z], F16, name="p5", tag="p5")
                    nc.gpsimd.tensor_mul(out=p5[:, :, :], in0=dzu[:, :, :],
                                         in1=wt[2][:, xsl, zc])
                    prods.append((p0, p1, p4, p5))
                return prods

            # ---- PE dy stage at KX granularity ----
            def stage_dy(c):
                """dy matmuls for chunk c: per channel one PSUM bank holding
                [w-side (KX planes) | u-side (KX planes)]."""
                u0 = 1 + KX * c
                hs = slice(u0, u0 + KX)
                dys = []
                for i in range(D):
                    dy = psum_dy.tile([Y, 2, KX, Z], F32, name="dy", tag="dy")
                    nc.tensor.matmul(dy[:, 0, :, :].rearrange("p a b -> p (a b)"),
                                     dyT, wt[i][:, hs, zc],
                                     start=True, stop=True)
                    nc.tensor.matmul(dy[:, 1, :, :].rearrange("p a b -> p (a b)"),
                                     ndyT, ut[i][:, hs, zc],
                                     start=True, stop=True)
                    dys.append(dy)
                return dys

            # ---- Pool dy-product stage at KX granularity ----
            def stage_dyprod(c, dys):
                u0 = 1 + KX * c
                xsl = slice(u0, u0 + KX)
                q23s = []
                for i in range(D):
                    p2 = ppool.tile([Y, KX, Z], F16, name="p2", tag="p2")
                    nc.gpsimd.tensor_mul(out=p2[:, :, :],
                                         in0=dys[i][:, 0, :, :],
                                         in1=ut[1][:, xsl, zc])
                    p3 = ppool.tile([Y, KX, Z], F16, name="p3", tag="p3")
                    nc.gpsimd.tensor_mul(out=p3[:, :, :],
                                         in0=dys[i][:, 1, :, :],
                                         in1=wt[1][:, xsl, zc])
                    q23 = ppool.tile([Y, KX, Z], F16, name="q23", tag="q23")
                    q_eng = nc.vector if i < q23_dve else nc.gpsimd
                    q_eng.tensor_add(out=q23[:, :, :], in0=p2[:, :, :],
                                     in1=p3[:, :, :])
                    q23s.append(q23)
                return q23s

            # ---- PE inject + Act evac + SP out-DMA at KX granularity ----
            def stage_inject(c, prods, q23s):
                u0 = 1 + KX * c
                xsl = slice(u0, u0 + KX)
                s = (KX * c) // kb
                h0 = KX * c - kb * s  # offset within super-chunk tiles
                hsl = slice(h0, h0 + KX)
                acc = psum_acc.tile([Y, D, KX, Z], F32, name="acc", tag="acc")
                for i in range(D):
                    p0, p1, p4, p5 = prods[i]
                    dst = acc[:, i, :, :].rearrange("p a b -> p (a b)")
                    nc.tensor.matmul(dst, eyeT, ut[i][:, xsl, zc],
                                     start=True, stop=False)
                    for k, src in enumerate((
                        p0[:, hsl, :], p1[:, hsl, :], q23s[i][:, :, :],
                        p4[:, hsl, :], p5[:, hsl, :],
                    )):
                        nc.tensor.matmul(dst, eyeT, src, start=False,
                                         stop=(k == 4))
                stage = spool.tile([Y, D, KX, Z], F16, name="stage", tag="stage")
                nc.scalar.copy(
                    out=stage[:, :, :, :].rearrange("p a b c -> p (a b c)"),
                    in_=acc[:, :, :, :].rearrange("p a b c -> p (a b c)"))
                x0 = KX * c
                nc.sync.dma_start(out=out_dram[:, x0 : x0 + KX, :],
                                  in_=stage[:, :, :, :])

            # ---- software pipeline ----
            # dy has a 2-chunk lookahead over injects so PE stays dense.
            chunks_per_super = kb // KX
            LOOK = 2
            prods_by_super = {}
            dys_by_chunk = {}
            q23_by_chunk = {}

            def ensure_super(s):
                if s < nsuper and s not in prods_by_super:
                    prods_by_super[s] = stage_diffs(s)

            def ensure_dy(c):
                if c < nchunk and c not in dys_by_chunk:
                    dys_by_chunk[c] = stage_dy(c)
                    q23_by_chunk[c] = stage_dyprod(c, dys_by_chunk[c])

            ensure_super(0)
            for c in range(LOOK):
                ensure_dy(c)
            for c in range(nchunk):
                ensure_super(((c + 1) * KX) // kb)
                ensure_dy(c + LOOK)
                s = (KX * c) // kb
                stage_inject(c, prods_by_super[s], q23_by_chunk.pop(c))
                dys_by_chunk.pop(c)

    if not nc.is_finalized():
        nc.finalize()
    return nc


def _host_shard(arr_b: np.ndarray, xs: int) -> list[np.ndarray]:
    """(D, X, Y, Z) f32 -> list over x-slabs of (D, Y, xs+2, ZP) fp16."""
    slabs = []
    for s in range(X // xs):
        idx = (np.arange(-1, xs + 1) + s * xs) % X
        sl = arr_b[:, idx, :, :]                  # (D, xs+2, Y, Z)
        sl = np.transpose(sl, (0, 2, 1, 3))       # (D, Y, xs+2, Z)
        sl = np.concatenate([sl[..., 127:128], sl, sl[..., 0:1]], axis=-1)
        slabs.append(np.ascontiguousarray(sl.astype(np.float16)))
    return slabs


def kernel(left: np.ndarray, right: np.ndarray) -> np.ndarray:
    left = np.asarray(left, dtype=np.float32)
    right = np.asarray(right, dtype=np.float32)
    assert left.shape == (B, D, X, Y, Z), left.shape

    u_full = left + right
    w_full = 0.125 * (left - right)

    wmats = _make_wmats()
    slabs_per_batch = X // XS  # 4

    ushards = [_host_shard(u_full[b], XS) for b in range(B)]
    wshards = [_host_shard(w_full[b], XS) for b in range(B)]

    maps = []
    for core in range(NCORES):
        b, s = divmod(core, slabs_per_batch)
        maps.append({
            "u": ushards[b][s],
            "w": wshards[b][s],
            "wmats": wmats,
        })

    nc = build_nc(XS)
    res = run_bass_kernel_spmd(nc, maps, core_ids=list(range(NCORES)))

    out = np.empty((B, D, X, Y, Z), dtype=np.float32)
    for core in range(NCORES):
        b, s = divmod(core, slabs_per_batch)
        o = res.results[core]["out"]              # (Y, D, XS, Z) fp16
        out[b, :, s * XS : (s + 1) * XS, :, :] = np.transpose(
            o.astype(np.float32), (1, 2, 0, 3))
    return out


# ---------------------------------------------------------------------------
# numpy reference of the same math (for probing without jax)
def _np_ref(left: np.ndarray, right: np.ndarray) -> np.ndarray:
    l = np.moveaxis(left, 1, -1).astype(np.float64)
    r = np.moveaxis(right, 1, -1).astype(np.float64)

    def jac(v):
        cols = []
        for j in range(3):
            ax = 1 + j
            g = (np.roll(v, -1, axis=ax) - np.roll(v, 1, axis=ax)) * 0.5
            cols.append(g)
        return np.stack(cols, axis=-1)

    jx, jy = jac(l), jac(r)
    br = np.einsum("bxyzij,bxyzj->bxyzi", jx, r) - np.einsum(
        "bxyzij,bxyzj->bxyzi", jy, l)
    z = l + r + 0.5 * br
    return np.moveaxis(z, -1, 1).astype(np.float32)


if __name__ == "__main__":
    import os
    probe_xs = int(os.environ.get("PROBE_XS", "32"))
    probe_cores = int(os.environ.get("PROBE_CORES", "1"))
    rng = np.random.default_rng(0)
    lf = rng.standard_normal((1, D, X, Y, Z), dtype=np.float32)
    rf = rng.standard_normal((1, D, X, Y, Z), dtype=np.float32)

    ush = _host_shard(lf[0] + rf[0], probe_xs)
    wsh = _host_shard(0.125 * (lf[0] - rf[0]), probe_xs)
    wm = _make_wmats()
    maps = [{"u": ush[c], "w": wsh[c], "wmats": wm}
            for c in range(probe_cores)]

    import time
    t0 = time.time()
    nc = build_nc(probe_xs)
    t1 = time.time()
    print(f"build: {t1-t0:.1f}s", flush=True)
    res = run_bass_kernel_spmd(nc, maps, core_ids=list(range(probe_cores)))
    t2 = time.time()
    print(f"compile+run: {t2-t1:.1f}s", flush=True)

    ref = _np_ref(lf, rf)
    for c in range(probe_cores):
        o = res.results[c]["out"]                 # (Y, D, xs, Z)
        o = np.transpose(o.astype(np.float32), (1, 2, 0, 3))
        expect = ref[0, :, c * probe_xs : (c + 1) * probe_xs]
        err = np.abs(o - expect)
        rel = np.linalg.norm(o - expect) / np.linalg.norm(expect)
        print(f"core {c}: rel={rel:.3e} absmax={err.max():.3e} "
              f"out_absmax={np.abs(expect).max():.3f}")


# revision 5
# speedup vs baseline: 1.0560x; 1.0560x over previous
"""Trainium2 Bass kernel for truncated BCH on 3D vector fields.

Math (matches the jax reference): with u = l + r, w = 0.125*(l - r):
  out_i = u_i + sum_j [ (D_j w_i) * u_j  +  (D~_j u_i) * w_j ]
where D_j v = v[.+1] - v[.-1] along spatial axis j (circulant wrap) and
D~ is the reversed diff (v[.-1] - v[.+1]), folding all signs so every
term is ADDED.  The 0.25 bracket scale is folded into w on the host
(0.125 per the u/w identity), so the device does no scaling at all.

Sharding: 8 cores = 2 batches x 4 X-slabs of 32 planes (+1 halo plane
each side, wrapped).  Host lays data per core as (D, Y, 34, 130) fp16
(z padded [z127 | z | z0]); output is (Y, D, 32, 128) fp16, cast to
fp32 on the host.

Pipeline: DVE computes diffs + x/z products one super-chunk (SUPERS
planes) AHEAD of a per-plane chain that runs dy matmuls (PE), dy
products straight out of PSUM (Pool), identity-weight PSUM injection
of all 7 terms (PE), and a PSUM->SBUF fp16 evacuation (Act).  Super
sizes shrink toward the end so the chain tail after DVE's last
product is short.
"""

import sys

sys.path.insert(0, "/opt/trn_rl_repo")

import numpy as np

import concourse.bass as bass
import concourse.bacc as bacc
import concourse.mybir as mybir
import concourse.tile as tile
from concourse.bass_utils import run_bass_kernel_spmd

B, D, X, Y, Z = 2, 3, 128, 128, 128
NCORES = 8
XS = (B * X) // NCORES  # 32 output x-planes per core
ZP = Z + 2              # z padded: [z127, z0..z127, z0]
SUPERS = [4, 14, 10, 4]  # x-planes per DVE diff/product super-chunk

F16 = mybir.dt.float16
F32 = mybir.dt.float32


def _make_wmats() -> np.ndarray:
    """[DyT | -DyT | I] as one (Y, 3Y) fp16 matrix (lhsT layout).

    matmul(out, lhsT, rhs) computes lhsT.T @ rhs.  We want Dy @ v with
    Dy[y, y'] = delta(y'=y+1) - delta(y'=y-1) (wrap), so lhsT = Dy.T.
    """
    e = np.eye(Y, dtype=np.float32)
    dy = np.roll(e, -1, axis=0) - np.roll(e, 1, axis=0)
    dyt = dy.T
    mats = np.concatenate([dyt, -dyt, e], axis=1)
    return mats.astype(np.float16)


def build_nc(xs: int = XS, *, supers=None, dzbufs: int = 6, dxbufs: int = 2,
             pbufs: int = 6, qbufs: int = 6, ybufs: int = 2, accbufs: int = 2,
             sbufs: int = 2, p4_dve=(0, 1), evac_pair: int = 2,
             ostage: int = 8) -> bass.Bass:
    xh = xs + 2
    if supers is None:
        supers = SUPERS if xs == XS else [min(4, xs)] + [4] * (xs // 4 - 1)
    assert sum(supers) == xs
    nc = bacc.Bacc(None)

    u_h = nc.declare_dram_parameter("u", [D, Y, xh, ZP], F16, isOutput=False)
    w_h = nc.declare_dram_parameter("w", [D, Y, xh, ZP], F16, isOutput=False)
    wm_h = nc.declare_dram_parameter("wmats", [Y, 3 * Y], F16, isOutput=False)
    out_h = nc.declare_dram_parameter("out", [Y, D, xs, Z], F16, isOutput=True)

    # (y d)-major view: dim0 = Y*D = 384, free = (x z) contiguous runs
    out_dram = out_h[:, :, :, :].rearrange("y d x z -> (y d) x z")

    with tile.TileContext(nc) as tc:
        with (
            tc.tile_pool(name="inp", bufs=1) as inp,
            tc.tile_pool(name="wp", bufs=1) as wp,
            tc.tile_pool(name="dzpool", bufs=dzbufs) as dzpool,
            tc.tile_pool(name="dxpool", bufs=dxbufs) as dxpool,
            tc.tile_pool(name="ppool", bufs=pbufs) as ppool,
            tc.tile_pool(name="qpool", bufs=qbufs) as qpool,
            tc.tile_pool(name="psum_dy", bufs=ybufs, space="PSUM") as psum_dy,
            tc.tile_pool(name="psum_acc", bufs=accbufs, space="PSUM") as psum_acc,
            tc.tile_pool(name="spool", bufs=sbufs) as spool,
        ):
            wt_m = wp.tile([Y, 3 * Y], F16, name="wt_m")
            nc.sync.dma_start(out=wt_m[:, :], in_=wm_h[:, :])
            dyT = wt_m[:, 0:Y]
            ndyT = wt_m[:, Y : 2 * Y]
            eyeT = wt_m[:, 2 * Y : 3 * Y]

            # Input x-splits aligned to super-chunk needs (super s needs
            # planes [1+start-1, 1+end+1)).  u loads on SP, w on Act.
            cuts = [0]
            acc_pl = 0
            for spl in supers[:-1]:
                acc_pl += spl
                cuts.append(min(acc_pl + 2, xh))
            cuts.append(xh)
            cuts = sorted(set(cuts))
            ut, wt = [], []
            for i in range(D):
                ut.append(inp.tile([Y, xh, ZP], F16, name=f"ut{i}", tag=f"ut{i}"))
            for i in range(D):
                wt.append(inp.tile([Y, xh, ZP], F16, name=f"wt{i}", tag=f"wt{i}"))
            for a, b2 in zip(cuts, cuts[1:]):
                for i in range(D):
                    nc.sync.dma_start(out=ut[i][:, a:b2, :],
                                      in_=u_h[i, :, a:b2, :])
                    nc.scalar.dma_start(out=wt[i][:, a:b2, :],
                                        in_=w_h[i, :, a:b2, :])

            zc = slice(1, 1 + Z)       # center z view
            zp1 = slice(2, 2 + Z)      # z+1
            zm1 = slice(0, 0 + Z)      # z-1

            # Prime PE's vector clock against every input DMA with tiny
            # matmuls, so real matmuls never need a second (DMA) wait —
            # TRN2 matmul instructions support a single sync wait.
            scratch = psum_acc.tile([8, 8], F32, name="scratch", tag="acc")
            for a in cuts[:-1]:
                for t in ut + wt:
                    nc.tensor.matmul(scratch[:, 0:1], wt_m[:, 0:8],
                                     t[:, a : a + 1, 0:1], start=True, stop=True)

            # ---- DVE super-chunk stage: diffs + x/z products ----
            def stage_diffs(s0, kb, sidx):
                u0 = 1 + s0
                xsl = slice(u0, u0 + kb)
                xp1 = slice(u0 + 1, u0 + 1 + kb)
                xm1 = slice(u0 - 1, u0 - 1 + kb)
                dz = []
                # dz diffs first: Pool's per-plane p4/p5 consume them early
                for i in range(D):
                    dzw = dzpool.tile([Y, kb, Z], F16, name="dzw", tag="dzw")
                    nc.vector.tensor_sub(out=dzw[:, :, :],
                                         in0=wt[i][:, xsl, zp1],
                                         in1=wt[i][:, xsl, zm1])
                    dzu = dzpool.tile([Y, kb, Z], F16, name="dzu", tag="dzu")
                    nc.vector.tensor_sub(out=dzu[:, :, :],
                                         in0=ut[i][:, xsl, zm1],
                                         in1=ut[i][:, xsl, zp1])
                    dz.append((dzw, dzu))
                prods = []
                for i in range(D):
                    dxw = dxpool.tile([Y, kb, Z], F16, name="dxw", tag="dxw")
                    nc.vector.tensor_sub(out=dxw[:, :, :],
                                         in0=wt[i][:, xp1, zc],
                                         in1=wt[i][:, xm1, zc])
                    dxu = dxpool.tile([Y, kb, Z], F16, name="dxu", tag="dxu")
                    nc.vector.tensor_sub(out=dxu[:, :, :],
                                         in0=ut[i][:, xm1, zc],
                                         in1=ut[i][:, xp1, zc])
                    p0 = ppool.tile([Y, kb, Z], F16, name="p0", tag="p0")
                    nc.vector.tensor_mul(out=p0[:, :, :], in0=dxw[:, :, :],
                                         in1=ut[0][:, xsl, zc])
                    p1 = ppool.tile([Y, kb, Z], F16, name="p1", tag="p1")
                    nc.vector.tensor_mul(out=p1[:, :, :], in0=dxu[:, :, :],
                                         in1=wt[0][:, xsl, zc])
                    prods.append([p0, p1, None, None])
                # p4 on DVE for the tuned channel subset; rest per-plane on Pool
                for i in range(D):
                    if i in p4_dve:
                        p4 = ppool.tile([Y, kb, Z], F16, name="p4", tag="p4")
                        nc.vector.tensor_mul(out=p4[:, :, :],
                                             in0=dz[i][0][:, :, :],
                                             in1=ut[2][:, xsl, zc])
                        prods[i][2] = p4
                return dz, prods

            # ---- per-plane chain: dy (PE) -> dy prods (Pool) -> inject ----
            def plane_dy(x):
                """dy matmuls for plane x (0-based output plane)."""
                hs = slice(1 + x, 2 + x)
                dy = psum_dy.tile([Y, D, 2, Z], F32, name="dy", tag="dy")
                for i in range(D):
                    nc.tensor.matmul(dy[:, i, 0, :], dyT, wt[i][:, hs, zc],
                                     start=True, stop=True)
                    nc.tensor.matmul(dy[:, i, 1, :], ndyT, ut[i][:, hs, zc],
                                     start=True, stop=True)
                return dy

            def plane_pool(x, dy, dz, s0):
                """Pool: dy products from PSUM + q23 + p4/p5 slices."""
                xsl = slice(1 + x, 2 + x)
                h = x - s0
                hsl = slice(h, h + 1)
                q23s = []
                for i in range(D):
                    p2 = qpool.tile([Y, 1, Z], F16, name="p2", tag="p2")
                    nc.gpsimd.tensor_mul(out=p2[:, :, :],
                                         in0=dy[:, i, 0:1, :],
                                         in1=ut[1][:, xsl, zc])
                    p3 = qpool.tile([Y, 1, Z], F16, name="p3", tag="p3")
                    nc.gpsimd.tensor_mul(out=p3[:, :, :],
                                         in0=dy[:, i, 1:2, :],
                                         in1=wt[1][:, xsl, zc])
                    q23 = qpool.tile([Y, 1, Z], F16, name="q23", tag="q23")
                    nc.gpsimd.tensor_add(out=q23[:, :, :], in0=p2[:, :, :],
                                         in1=p3[:, :, :])
                    q23s.append(q23)
                p45 = []
                for i in range(D):
                    dzw, dzu = dz[i]
                    if i in p4_dve:
                        p4v = None  # comes from the DVE super stage
                    else:
                        p4 = qpool.tile([Y, 1, Z], F16, name="p4p", tag="p4p")
                        nc.gpsimd.tensor_mul(out=p4[:, :, :],
                                             in0=dzw[:, hsl, :],
                                             in1=ut[2][:, xsl, zc])
                        p4v = p4[:, :, :]
                    p5 = qpool.tile([Y, 1, Z], F16, name="p5", tag="p5")
                    nc.gpsimd.tensor_mul(out=p5[:, :, :],
                                         in0=dzu[:, hsl, :],
                                         in1=wt[2][:, xsl, zc])
                    p45.append((p4v, p5))
                return q23s, p45

            def plane_inject(x, prods, q23s, p45, s0, acc, slot):
                xsl = slice(1 + x, 2 + x)
                h = x - s0
                hsl = slice(h, h + 1)
                for i in range(D):
                    p0, p1, p4, _ = prods[i]
                    p4v = p45[i][0] if p4 is None else p4[:, hsl, :]
                    dst = acc[:, i, slot, :]
                    nc.tensor.matmul(dst, eyeT, ut[i][:, xsl, zc],
                                     start=True, stop=False)
                    for k, src in enumerate((
                        p0[:, hsl, :], p1[:, hsl, :], q23s[i][:, :, :],
                        p4v, p45[i][1][:, :, :],
                    )):
                        nc.tensor.matmul(dst, eyeT, src, start=False,
                                         stop=(k == 4))

            # ---- emission: DVE one super ahead of the per-plane chain ----
            sstarts = []
            s0 = 0
            for kb in supers:
                sstarts.append(s0)
                s0 += kb
            dz_s, prods_s = stage_diffs(sstarts[0], supers[0], 0)
            acc = None
            stage = None
            acc_x0 = 0
            stage_x0 = 0
            for sidx, kb in enumerate(supers):
                if sidx + 1 < len(supers):
                    dz_n, prods_n = stage_diffs(sstarts[sidx + 1],
                                                supers[sidx + 1], sidx + 1)
                else:
                    dz_n = prods_n = None
                for x in range(sstarts[sidx], sstarts[sidx] + kb):
                    dy = plane_dy(x)
                    q23s, p45 = plane_pool(x, dy, dz_s, sstarts[sidx])
                    if acc is None:
                        acc = psum_acc.tile([Y, D, evac_pair, Z], F32,
                                            name="acc", tag="acc")
                        acc_x0 = x
                    if stage is None:
                        stage = spool.tile([Y, D, ostage, Z], F16,
                                           name="stage", tag="stage")
                        stage_x0 = x
                    plane_inject(x, prods_s, q23s, p45, sstarts[sidx],
                                 acc, x - acc_x0)
                    if x - acc_x0 == evac_pair - 1:
                        nc.scalar.copy(
                            out=stage[:, :, acc_x0 - stage_x0 :
                                      acc_x0 - stage_x0 + evac_pair, :],
                            in_=acc[:, :, :, :])
                        acc = None
                    if x - stage_x0 == ostage - 1:
                        nc.sync.dma_start(
                            out=out_dram[:, stage_x0 : stage_x0 + ostage, :],
                            in_=stage[:, :, :, :])
                        stage = None
                dz_s, prods_s = dz_n, prods_n

    if not nc.is_finalized():
        nc.finalize()
    return nc


def _host_shard(arr_b: np.ndarray, xs: int) -> list[np.ndarray]:
    """(D, X, Y, Z) f32 -> list over x-slabs of (D, Y, xs+2, ZP) fp16."""
    slabs = []
    for s in range(X // xs):
        idx = (np.arange(-1, xs + 1) + s * xs) % X
        sl = arr_b[:, idx, :, :]                  # (D, xs+2, Y, Z)
        sl = np.transpose(sl, (0, 2, 1, 3))       # (D, Y, xs+2, Z)
        sl = np.concatenate([sl[..., 127:128], sl, sl[..., 0:1]], axis=-1)
        slabs.append(np.ascontiguousarray(sl.astype(np.float16)))
    return slabs


def kernel(left: np.ndarray, right: np.ndarray) -> np.ndarray:
    left = np.asarray(left, dtype=np.float32)
    right = np.asarray(right, dtype=np.float32)
    assert left.shape == (B, D, X, Y, Z), left.shape

    u_full = left + right
    w_full = 0.125 * (left - right)

    wmats = _make_wmats()
    slabs_per_batch = X // XS  # 4

    ushards = [_host_shard(u_full[b], XS) for b in range(B)]
    wshards = [_host_shard(w_full[b], XS) for b in range(B)]

    maps = []
    for core in range(NCORES):
        b, s = divmod(core, slabs_per_batch)
        maps.append({
            "u": ushards[b][s],
            "w": wshards[b][s],
            "wmats": wmats,
        })

    nc = build_nc(XS)
    res = run_bass_kernel_spmd(nc, maps, core_ids=list(range(NCORES)))

    out = np.empty((B, D, X, Y, Z), dtype=np.float32)
    for core in range(NCORES):
        b, s = divmod(core, slabs_per_batch)
        o = res.results[core]["out"]              # (Y, D, XS, Z) fp16
        out[b, :, s * XS : (s + 1) * XS, :, :] = np.transpose(
            o.astype(np.float32), (1, 2, 0, 3))
    return out


# ---------------------------------------------------------------------------
# numpy reference of the same math (for probing without jax)
def _np_ref(left: np.ndarray, right: np.ndarray) -> np.ndarray:
    l = np.moveaxis(left, 1, -1).astype(np.float64)
    r = np.moveaxis(right, 1, -1).astype(np.float64)

    def jac(v):
        cols = []
        for j in range(3):
            ax = 1 + j
            g = (np.roll(v, -1, axis=ax) - np.roll(v, 1, axis=ax)) * 0.5
            cols.append(g)
        return np.stack(cols, axis=-1)

    jx, jy = jac(l), jac(r)
    br = np.einsum("bxyzij,bxyzj->bxyzi", jx, r) - np.einsum(
        "bxyzij,bxyzj->bxyzi", jy, l)
    z = l + r + 0.5 * br
    return np.moveaxis(z, -1, 1).astype(np.float32)


if __name__ == "__main__":
    import os
    probe_xs = int(os.environ.get("PROBE_XS", "32"))
    probe_cores = int(os.environ.get("PROBE_CORES", "1"))
    rng = np.random.default_rng(0)
    lf = rng.standard_normal((1, D, X, Y, Z), dtype=np.float32)
    rf = rng.standard_normal((1, D, X, Y, Z), dtype=np.float32)

    ush = _host_shard(lf[0] + rf[0], probe_xs)
    wsh = _host_shard(0.125 * (lf[0] - rf[0]), probe_xs)
    wm = _make_wmats()
    maps = [{"u": ush[c], "w": wsh[c], "wmats": wm}
            for c in range(probe_cores)]

    import time
    t0 = time.time()
    nc = build_nc(probe_xs)
    t1 = time.time()
    print(f"build: {t1-t0:.1f}s", flush=True)
    res = run_bass_kernel_spmd(nc, maps, core_ids=list(range(probe_cores)))
    t2 = time.time()
    print(f"compile+run: {t2-t1:.1f}s", flush=True)

    ref = _np_ref(lf, rf)
    for c in range(probe_cores):
        o = res.results[c]["out"]                 # (Y, D, xs, Z)
        o = np.transpose(o.astype(np.float32), (1, 2, 0, 3))
        expect = ref[0, :, c * probe_xs : (c + 1) * probe_xs]
        err = np.abs(o - expect)
        rel = np.linalg.norm(o - expect) / np.linalg.norm(expect)
        print(f"core {c}: rel={rel:.3e} absmax={err.max():.3e} "
              f"out_absmax={np.abs(expect).max():.3f}")


# revision 6
# speedup vs baseline: 1.2515x; 1.1851x over previous
"""Trainium2 Bass kernel for truncated BCH on 3D vector fields.

Math (matches the jax reference): with u = l + r, w = 0.125*(l - r):
  out_i = u_i + sum_j [ (D_j w_i) * u_j  +  (D~_j u_i) * w_j ]
where D_j v = v[.+1] - v[.-1] along spatial axis j (circulant wrap) and
D~ is the reversed diff (v[.-1] - v[.+1]); all signs fold so every term
is ADDED and the 0.25 bracket scale lives in w (host-folded, 0.125 per
the u/w identity) — the device never scales anything.

Sharding: 8 cores = 2 batches x 4 X-slabs of 32 planes (+1 halo plane on
each side, wrapped).  Host re-lays data per core as (D, Y, X_halo, Z_pad)
fp16 so that SBUF partition dim = Y and all DMA runs are long/contiguous.
Output is (Y, D, 32, 128) fp16 ((y d)-major so the store DMA presents a
384-row access pattern), cast back to fp32 on the host.

Per-core engine split:
  - TensorE : Y-diffs as circulant shift-difference matmuls (lhsT = DyT
              for the w-side, -DyT for the u-side), plus accumulation of
              6 terms (u, p0, p1, q23, p4, p5) via identity-weight
              matmuls into a PSUM accumulator.
  - VectorE : X-diffs, Z-diffs (w-side), products p0/p1/p2, q23 pre-sum.
  - GPSIMD  : Z-diffs (u-side), p3 straight out of the u-side dy PSUM
              (no evacuation), p4, p5.
  - ScalarE : evacuates the w-side Y-diff PSUM half to SBUF fp16, the
              final PSUM->fp16 evacuation, and the w input DMAs.
"""

import sys

sys.path.insert(0, "/opt/trn_rl_repo")

import numpy as np

import concourse.bass as bass
import concourse.bacc as bacc
import concourse.mybir as mybir
import concourse.tile as tile
from concourse.bass_utils import run_bass_kernel_spmd

B, D, X, Y, Z = 2, 3, 128, 128, 128
NCORES = 8
XS = (B * X) // NCORES  # 32 output x-planes per core
ZP = Z + 2              # z padded: [z127, z0..z127, z0]
KX = 4                  # x-planes per compute chunk (psum bank = 512 f32)
# (kb, kx) work items for xs=32; kb multiple of kx; small tail items
SIZES = [(4, 4), (8, 4), (8, 4), (8, 4), (2, 2), (2, 2)]

F16 = mybir.dt.float16
F32 = mybir.dt.float32


def _make_wmats() -> np.ndarray:
    """[DyT | -DyT | I] as one (Y, 3Y) fp16 matrix (lhsT layout).

    matmul(out, lhsT, rhs) computes lhsT.T @ rhs.  We want Dy @ v with
    Dy[y, y'] = delta(y'=y+1) - delta(y'=y-1) (wrap), so lhsT = Dy.T.
    """
    e = np.eye(Y, dtype=np.float32)
    dy = np.roll(e, -1, axis=0) - np.roll(e, 1, axis=0)
    dyt = dy.T
    mats = np.concatenate([dyt, -dyt, e], axis=1)
    return mats.astype(np.float16)


def build_nc(xs: int = XS, *, dbufs: int = 3, pbufs: int = 3, ylbufs: int = 2,
             accbufs: int = 4, sbufs: int = 3) -> bass.Bass:
    xh = xs + 2
    nc = bacc.Bacc(None)

    u_h = nc.declare_dram_parameter("u", [D, Y, xh, ZP], F16, isOutput=False)
    w_h = nc.declare_dram_parameter("w", [D, Y, xh, ZP], F16, isOutput=False)
    wm_h = nc.declare_dram_parameter("wmats", [Y, 3 * Y], F16, isOutput=False)
    out_h = nc.declare_dram_parameter("out", [Y, D, xs, Z], F16, isOutput=True)

    # (y d)-major view: dim0 = Y*D = 384, free = contiguous (x z) runs
    out_dram = out_h[:, :, :, :].rearrange("y d x z -> (y d) x z")

    with tile.TileContext(nc) as tc:
        with (
            tc.tile_pool(name="inp", bufs=1) as inp,
            tc.tile_pool(name="wp", bufs=1) as wp,
            tc.tile_pool(name="dpool", bufs=dbufs) as dpool,
            tc.tile_pool(name="ppool", bufs=pbufs) as ppool,
            tc.tile_pool(name="psum_dy", bufs=ylbufs, space="PSUM") as psum_dy,
            tc.tile_pool(name="psum_acc", bufs=accbufs, space="PSUM") as psum_acc,
            tc.tile_pool(name="spool", bufs=sbufs) as spool,
        ):
            wt_m = wp.tile([Y, 3 * Y], F16, name="wt_m")
            nc.sync.dma_start(out=wt_m[:, :], in_=wm_h[:, :])
            dyT = wt_m[:, 0:Y]
            ndyT = wt_m[:, Y : 2 * Y]
            eyeT = wt_m[:, 2 * Y : 3 * Y]

            # Load each channel in x-splits so early chunks can start while
            # the rest streams in (Tile tracks subtile deps).  u loads ride
            # the SP queue, w loads the Act queue (2 parallel DMA queues).
            cuts = [0, 6, 14, 22, 30, xh] if xh >= 32 else [0, xh]
            ut, wt = [], []
            for i in range(D):
                ut.append(inp.tile([Y, xh, ZP], F16, name=f"ut{i}", tag=f"ut{i}"))
            for i in range(D):
                wt.append(inp.tile([Y, xh, ZP], F16, name=f"wt{i}", tag=f"wt{i}"))
            for a, b2 in zip(cuts, cuts[1:]):
                for i in range(D):
                    nc.sync.dma_start(out=ut[i][:, a:b2, :],
                                      in_=u_h[i, :, a:b2, :])
                    nc.scalar.dma_start(out=wt[i][:, a:b2, :],
                                        in_=w_h[i, :, a:b2, :])

            zc = slice(1, 1 + Z)       # center z view
            zp1 = slice(2, 2 + Z)      # z+1
            zm1 = slice(0, 0 + Z)      # z-1

            # Prime PE's vector clock against every input DMA with tiny
            # matmuls, so real matmuls never need a second (DMA) wait —
            # TRN2 matmul instructions support a single sync wait.
            scratch = psum_acc.tile([8, 8], F32, name="scratch", tag="acc")
            for a in cuts[:-1]:
                for t in ut + wt:
                    nc.tensor.matmul(scratch[:, 0:1], wt_m[:, 0:8],
                                     t[:, a : a + 1, 0:1], start=True, stop=True)

            # work items of (x0, kb, kx) planes: small final items keep the
            # pipeline drain short; big middle items amortize DVE overhead.
            if xs == 32:
                sizes = SIZES
            else:
                sizes = [(KX, KX)] * (xs // KX)
            items = []
            off = 0
            for sz, kx in sizes:
                items.append((off, sz, kx))
                off += sz
            assert off == xs

            def stage_a(idx, item):
                """diffs + products for work item (x0, kb planes)."""
                x0, kb, kx = item
                u0 = 1 + x0
                KB = kb
                xsl = slice(u0, u0 + KB)
                xp1 = slice(u0 + 1, u0 + 1 + KB)
                xm1 = slice(u0 - 1, u0 - 1 + KB)
                chunk = []
                ylrs = []
                for i in range(D):
                    # Y diffs on PE into one PSUM tile per kx chunk:
                    # w-side in the first half (evacuated to fp16 by ScalarE),
                    # u-side in the second half (consumed in-place by GPSIMD).
                    nh = KB // kx
                    dyw = dpool.tile([Y, nh, kx, Z], F16, name="dyw", tag="dyw")
                    ylr_ch = []
                    for h in range(nh):
                        hs = slice(u0 + kx * h, u0 + kx * h + kx)
                        ylr = psum_dy.tile([Y, 2, kx, Z], F32,
                                           name="ylr", tag="ylr")
                        nc.tensor.matmul(
                            ylr[:, 0, :, :].rearrange("p a b -> p (a b)"),
                            dyT, wt[i][:, hs, zc], start=True, stop=True)
                        nc.tensor.matmul(
                            ylr[:, 1, :, :].rearrange("p a b -> p (a b)"),
                            ndyT, ut[i][:, hs, zc], start=True, stop=True)
                        nc.scalar.copy(
                            out=dyw[:, h, :, :]
                                .rearrange("p a b -> p (a b)"),
                            in_=ylr[:, 0, :, :].rearrange("p a b -> p (a b)"))
                        ylr_ch.append(ylr)
                    ylrs.append(ylr_ch)

                    # X diffs on DVE (u-side reversed for sign fold)
                    dxw = dpool.tile([Y, KB, Z], F16, name="dxw", tag="dxw")
                    nc.vector.tensor_sub(out=dxw[:, :, :],
                                         in0=wt[i][:, xp1, zc],
                                         in1=wt[i][:, xm1, zc])
                    dxu = dpool.tile([Y, KB, Z], F16, name="dxu", tag="dxu")
                    nc.vector.tensor_sub(out=dxu[:, :, :],
                                         in0=ut[i][:, xm1, zc],
                                         in1=ut[i][:, xp1, zc])

                    # Z diffs: w-side on DVE, u-side on GPSIMD
                    dzw = dpool.tile([Y, KB, Z], F16, name="dzw", tag="dzw")
                    nc.vector.tensor_sub(out=dzw[:, :, :],
                                         in0=wt[i][:, xsl, zp1],
                                         in1=wt[i][:, xsl, zm1])
                    dzu = dpool.tile([Y, KB, Z], F16, name="dzu", tag="dzu")
                    nc.gpsimd.tensor_sub(out=dzu[:, :, :],
                                         in0=ut[i][:, xsl, zm1],
                                         in1=ut[i][:, xsl, zp1])

                    # products
                    p0 = ppool.tile([Y, KB, Z], F16, name="p0", tag="p0")
                    nc.vector.tensor_mul(out=p0[:, :, :], in0=dxw[:, :, :],
                                         in1=ut[0][:, xsl, zc])
                    p1 = ppool.tile([Y, KB, Z], F16, name="p1", tag="p1")
                    nc.vector.tensor_mul(out=p1[:, :, :], in0=dxu[:, :, :],
                                         in1=wt[0][:, xsl, zc])
                    p2 = ppool.tile([Y, KB, Z], F16, name="p2", tag="p2")
                    nc.vector.tensor_mul(
                        out=p2[:, :, :],
                        in0=dyw[:, :, :, :].rearrange("p a b c -> p (a b) c"),
                        in1=ut[1][:, xsl, zc])
                    # p3 from the u-side dy PSUM, per kx chunk, on GPSIMD;
                    # q23 = p2 + p3 on DVE
                    q23 = ppool.tile([Y, KB, Z], F16, name="q23", tag="q23")
                    for h in range(nh):
                        hs2 = slice(u0 + kx * h, u0 + kx * h + kx)
                        hb = slice(kx * h, kx * h + kx)
                        p3 = ppool.tile([Y, kx, Z], F16, name="p3", tag="p3")
                        nc.gpsimd.tensor_mul(out=p3[:, :, :],
                                             in0=ylr_ch[h][:, 1, :, :],
                                             in1=wt[1][:, hs2, zc])
                        nc.vector.tensor_add(out=q23[:, hb, :],
                                             in0=p2[:, hb, :],
                                             in1=p3[:, :, :])
                    p4 = ppool.tile([Y, KB, Z], F16, name="p4", tag="p4")
                    nc.gpsimd.tensor_mul(out=p4[:, :, :], in0=dzw[:, :, :],
                                         in1=ut[2][:, xsl, zc])
                    p5 = ppool.tile([Y, KB, Z], F16, name="p5", tag="p5")
                    nc.gpsimd.tensor_mul(out=p5[:, :, :], in0=dzu[:, :, :],
                                         in1=wt[2][:, xsl, zc])
                    chunk.append((p0, p1, q23, p4, p5))
                return chunk

            def stage_b(item, chunk, split_dma=False):
                """PSUM accumulation + evac + DMA out (per kx chunk)."""
                x0i, kb, kx = item
                stages = [spool.tile([Y, D, kx, Z], F16, name="stage",
                                     tag="stage") for _ in range(kb // kx)]
                for i in range(D):
                    for h in range(kb // kx):
                        stage = stages[h]
                        p0, p1, q23, p4, p5 = chunk[i]
                        xsl = slice(1 + x0i + kx * h, 1 + x0i + kx * h + kx)
                        hb = slice(kx * h, kx * h + kx)
                        acc = psum_acc.tile([Y, kx * Z], F32, name="acc",
                                            tag="acc")
                        nc.tensor.matmul(
                            acc[:, :], eyeT,
                            ut[i][:, xsl, zc], start=True, stop=False)
                        for k, p in enumerate((p0, p1, q23, p4, p5)):
                            nc.tensor.matmul(
                                acc[:, :], eyeT,
                                p[:, hb, :].rearrange("p a b -> p (a b)"),
                                start=False, stop=(k == 4),
                            )
                        nc.scalar.copy(
                            out=stage[:, i, :, :].rearrange("p a b -> p (a b)"),
                            in_=acc[:, :],
                        )
                        if split_dma:
                            x0 = x0i + kx * h
                            nc.sync.dma_start(
                                out=out_dram[:, x0 : x0 + kx, :],
                                in_=stage[:, i : i + 1, :, :]
                                    .rearrange("p a b c -> (p a) b c"),
                            )
                if not split_dma:
                    for h in range(kb // kx):
                        x0 = x0i + kx * h
                        nc.sync.dma_start(
                            out=out_dram[:, x0 : x0 + kx, :],
                            in_=stages[h][:, :, :, :],
                        )

            # software pipeline: A(0), A(1), B(0), A(2), B(1), ... B(last)
            prev = None
            prev_chunk = None
            for idx, item in enumerate(items):
                ch = stage_a(idx, item)
                if prev is not None:
                    stage_b(prev, prev_chunk)
                prev, prev_chunk = item, ch
            stage_b(prev, prev_chunk)

    if not nc.is_finalized():
        nc.finalize()
    return nc


def _host_shard(arr_b: np.ndarray, xs: int) -> list[np.ndarray]:
    """(D, X, Y, Z) f32 -> list over x-slabs of (D, Y, xs+2, ZP) fp16."""
    slabs = []
    for s in range(X // xs):
        idx = (np.arange(-1, xs + 1) + s * xs) % X
        sl = arr_b[:, idx, :, :]                  # (D, xs+2, Y, Z)
        sl = np.transpose(sl, (0, 2, 1, 3))       # (D, Y, xs+2, Z)
        sl = np.concatenate([sl[..., 127:128], sl, sl[..., 0:1]], axis=-1)
        slabs.append(np.ascontiguousarray(sl.astype(np.float16)))
    return slabs


def kernel(left: np.ndarray, right: np.ndarray) -> np.ndarray:
    left = np.asarray(left, dtype=np.float32)
    right = np.asarray(right, dtype=np.float32)
    assert left.shape == (B, D, X, Y, Z), left.shape

    u_full = left + right
    w_full = 0.125 * (left - right)

    wmats = _make_wmats()
    slabs_per_batch = X // XS  # 4

    ushards = [_host_shard(u_full[b], XS) for b in range(B)]
    wshards = [_host_shard(w_full[b], XS) for b in range(B)]

    maps = []
    for core in range(NCORES):
        b, s = divmod(core, slabs_per_batch)
        maps.append({
            "u": ushards[b][s],
            "w": wshards[b][s],
            "wmats": wmats,
        })

    nc = build_nc(XS)
    res = run_bass_kernel_spmd(nc, maps, core_ids=list(range(NCORES)))

    out = np.empty((B, D, X, Y, Z), dtype=np.float32)
    for core in range(NCORES):
        b, s = divmod(core, slabs_per_batch)
        o = res.results[core]["out"]              # (Y, D, XS, Z) fp16
        out[b, :, s * XS : (s + 1) * XS, :, :] = np.transpose(
            o.astype(np.float32), (1, 2, 0, 3))
    return out


# ---------------------------------------------------------------------------
# numpy reference of the same math (for probing without jax)
def _np_ref(left: np.ndarray, right: np.ndarray) -> np.ndarray:
    l = np.moveaxis(left, 1, -1).astype(np.float64)
    r = np.moveaxis(right, 1, -1).astype(np.float64)

    def jac(v):
        cols = []
        for j in range(3):
            ax = 1 + j
            g = (np.roll(v, -1, axis=ax) - np.roll(v, 1, axis=ax)) * 0.5
            cols.append(g)
        return np.stack(cols, axis=-1)

    jx, jy = jac(l), jac(r)
    br = np.einsum("bxyzij,bxyzj->bxyzi", jx, r) - np.einsum(
        "bxyzij,bxyzj->bxyzi", jy, l)
    z = l + r + 0.5 * br
    return np.moveaxis(z, -1, 1).astype(np.float32)


if __name__ == "__main__":
    import os
    probe_xs = int(os.environ.get("PROBE_XS", "8"))
    probe_cores = int(os.environ.get("PROBE_CORES", "1"))
    rng = np.random.default_rng(0)
    lf = rng.standard_normal((1, D, X, Y, Z), dtype=np.float32)
    rf = rng.standard_normal((1, D, X, Y, Z), dtype=np.float32)

    ush = _host_shard(lf[0] + rf[0], probe_xs)
    wsh = _host_shard(0.125 * (lf[0] - rf[0]), probe_xs)
    wm = _make_wmats()
    maps = [{"u": ush[c], "w": wsh[c], "wmats": wm}
            for c in range(probe_cores)]

    import time
    t0 = time.time()
    nc = build_nc(probe_xs)
    t1 = time.time()
    print(f"build: {t1-t0:.1f}s", flush=True)
    res = run_bass_kernel_spmd(nc, maps, core_ids=list(range(probe_cores)))
    t2 = time.time()
    print(f"compile+run: {t2-t1:.1f}s", flush=True)

    ref = _np_ref(lf, rf)
    for c in range(probe_cores):
        o = res.results[c]["out"]                 # (Y, D, xs, Z)
        o = np.transpose(o.astype(np.float32), (1, 2, 0, 3))
        expect = ref[0, :, c * probe_xs : (c + 1) * probe_xs]
        err = np.abs(o - expect)
        rel = np.linalg.norm(o - expect) / np.linalg.norm(expect)
        print(f"core {c}: rel={rel:.3e} absmax={err.max():.3e} "
              f"out_absmax={np.abs(expect).max():.3f}")
